# revision 1
# baseline (speedup 1.0000x reference)
"""Trainium2 Bass kernel for nn_Circuit (batch quantum circuit simulation).

Math: the circuit is u1 on every qubit, u2 on disjoint adjacent pairs, u1 on
every qubit again.  Since all gates factor over the 10 disjoint qubit pairs,
the whole circuit equals M^{tensor 10} with M = (u1 x u1) @ u2 @ (u1 x u1),
a single 4x4 complex matrix applied to every base-4 digit of the state index.

Strategy (data-parallel, one batch item per core):
  State as real fp32 [128, 16384]; partition bits = (c | 3 digits) where c is
  the re/im component bit.  Three matmul passes with realified 128x128
  stationaries contract digits 3..6 (super-pass with d6 blocked in the free
  dim), 7..9, and 0..2; two full 128x128 PE-transpose exchanges move digit
  groups between partitions and free dim; c alternates partition/free and the
  middle pass mixes re/im via PSUM accumulation over component slices.
  All matmuls/transposes in fp32r (TF32); PSUM accumulation fp32.
"""
import numpy as np

import concourse.bacc as bacc
import concourse.tile as tile
import concourse.mybir as mybir

F32 = mybir.dt.float32
F32R = mybir.dt.float32r

NQ = 20
BATCH = 8
DIM = 2 ** NQ
N_CORES = 8

_NC_CACHE = {}


def _realify(W):
    """lhsT [128,128] for out[(c',w)] = sum_{c,v} L[(c,v),(c',w)] x[(c,v)],
    complex W acting as out[w] = sum_v W[w,v] x[v]."""
    Wr, Wi = W.real, W.imag
    L = np.zeros((128, 128), np.float64)
    L[0:64, 0:64] = Wr.T
    L[64:128, 0:64] = -Wi.T
    L[0:64, 64:128] = Wi.T
    L[64:128, 64:128] = Wr.T
    return L


def build_weights(u1_re, u1_im, u2_re, u2_im):
    """Host-side: fused gate M, Kronecker powers, realified stationaries.
    Returns [128, 22*128] fp32 (22 matrices packed side by side)."""
    u1 = np.asarray(u1_re, np.float64) + 1j * np.asarray(u1_im, np.float64)
    u2 = np.asarray(u2_re, np.float64) + 1j * np.asarray(u2_im, np.float64)
    A = np.kron(u1, u1)
    M = A @ u2 @ A                      # 4x4 complex
    W3 = np.kron(M, np.kron(M, M))      # 64x64 complex, digit-major

    mats = []
    # 0..15: pass A stationaries, blocks (j = d6 in, i = e6 out), m = j*4+i
    for j in range(4):
        for i in range(4):
            mats.append(_realify(M[i, j] * W3))
    # 16..19: pass B stationaries SB[c][c'] = kron(I2, C_cc'.T), m = 16+c*2+c'
    Wr, Wi = W3.real, W3.imag
    C = {(0, 0): Wr, (0, 1): Wi, (1, 0): -Wi, (1, 1): Wr}
    for c in range(2):
        for cp in range(2):
            mats.append(np.kron(np.eye(2), C[(c, cp)].T))
    # 20: pass C stationary
    mats.append(_realify(W3))
    # 21: identity for PE transposes
    mats.append(np.eye(128))

    wts = np.stack(mats)                                  # [22,128,128]
    packed = wts.transpose(1, 0, 2).reshape(128, 22 * 128)
    return np.ascontiguousarray(packed).astype(np.float32)


def build_nc(repeat=1):
    nc = bacc.Bacc("TRN2", target_bir_lowering=False, debug=False,
                   num_devices=N_CORES)
    xin_d = nc.dram_tensor("xin", [128, 16384], F32R, kind="ExternalInput").ap()
    wts_d = nc.dram_tensor("wts", [128, 22 * 128], F32R,
                           kind="ExternalInput").ap()
    xout_d = nc.dram_tensor("xout", [128, 16384], F32,
                            kind="ExternalOutput").ap()

    dcnt = [0]

    with tile.TileContext(nc) as tc:
        with tc.tile_pool(name="sb", bufs=1) as sb, \
             tc.tile_pool(name="ps", bufs=4, space="PSUM") as ps:

            wt = sb.tile([128, 22 * 128], F32R, tag="wt")
            nc.gpsimd.dma_start(wt[:], wts_d)

            def W(m):
                return wt[:, m * 128:(m + 1) * 128]

            b1 = [sb.tile([128, 2048], F32R, tag=f"b1_{k}", name=f"b1_{k}")
                  for k in range(8)]
            b2 = [sb.tile([128, 2048], F32R, tag=f"b2_{k}", name=f"b2_{k}")
                  for k in range(8)]
            bC = sb.tile([128, 16384], F32R, tag="bC")

            for k in range(8):
                eng = nc.sync if k % 2 == 0 else nc.scalar
                eng.dma_start(b1[k][:], xin_d[:, 2048 * k:2048 * (k + 1)])

            def drain(o, i):
                if dcnt[0] % 2 == 0:
                    nc.scalar.copy(o, i)
                else:
                    nc.vector.tensor_copy(o, i)
                dcnt[0] += 1

            for _rep in range(repeat):
                # ---- pass A: contract d3,d4,d5 (partitions) + d6 (free blocks)
                # b1 layout L_A: f = v012*256 + d6*64 + r789
                # out -> b2 layout L_B: f = v012*256 + e6*64 + r789
                for ck in range(8):
                    rhsv = b1[ck][:].rearrange("p (v j r) -> p j v r",
                                               v=8, j=4, r=64)
                    outv = b2[ck][:].rearrange("p (v e r) -> p e v r",
                                               v=8, e=4, r=64)
                    for ih in range(2):
                        pt = ps.tile([128, 1024], F32, tag="ps")
                        for il in range(2):
                            i = 2 * ih + il
                            for j in range(4):
                                nc.tensor.matmul(pt[:, il * 512:(il + 1) * 512],
                                                 W(j * 4 + i), rhsv[:, j],
                                                 start=(j == 0), stop=(j == 3))
                        inv = pt[:].rearrange("p (e v r) -> p e v r",
                                              e=2, v=8, r=64)
                        drain(outv[:, 2 * ih:2 * ih + 2], inv)

                # ---- E1: full transposes; in-cols = (e6l, r789) contiguous 128
                # out -> bC layout L_C: f = w345*256 + e6h*128 + c*64 + v012
                bCo = bC[:].rearrange("p (w ec v) -> p v ec w",
                                      w=64, ec=4, v=64)
                for g in range(8):
                    for vh in range(2):
                        pt = ps.tile([128, 1024], F32, tag="ps")
                        for v4 in range(4):
                            v8 = 4 * vh + v4
                            for e6h in range(2):
                                bidx = v4 * 2 + e6h
                                off = v8 * 256 + e6h * 128
                                nc.tensor.transpose(
                                    pt[:, bidx * 128:(bidx + 1) * 128]
                                    .bitcast(F32R),
                                    b2[g][:, off:off + 128], W(21))
                        inv = pt[:].rearrange("p (v ec w) -> p v ec w",
                                              v=4, ec=4, w=64)
                        base = 8 * g + 4 * vh
                        drain(bCo[:, base:base + 4], inv)

                # ---- pass B: contract d7,d8,d9 (partitions (e6l, r789)),
                # c in free; accumulate over c slices, c' -> free bit.
                # out -> b1 layout L_D: f = w345*256 + e6h*128 + c'*64 + v012
                bCv = bC[:].rearrange("p (t we c v) -> p c t we v",
                                      t=16, we=8, c=2, v=64)
                for q in range(8):
                    outv = b1[q][:].rearrange("p (twe c v) -> p c twe v",
                                              twe=16, c=2, v=64)
                    for cp in range(2):
                        pt = ps.tile([128, 1024], F32, tag="ps")
                        for tl in range(2):
                            tp = 2 * q + tl
                            for c in range(2):
                                nc.tensor.matmul(
                                    pt[:, tl * 512:(tl + 1) * 512],
                                    W(16 + c * 2 + cp), bCv[:, c, tp],
                                    start=(c == 0), stop=(c == 1))
                        inv = pt[:].rearrange("p (twe v) -> p twe v",
                                              twe=16, v=64)
                        drain(outv[:, cp], inv)

                # ---- E2: full transposes; in-cols = (c', v012) contiguous 128
                # out -> b2 layout L_E: f = w345*256 + e6*64 + w789 (plain copy)
                for q in range(8):
                    for wh in range(2):
                        pt = ps.tile([128, 1024], F32, tag="ps")
                        for w4 in range(4):
                            for e6h in range(2):
                                bidx = w4 * 2 + e6h
                                off = (4 * wh + w4) * 256 + e6h * 128
                                nc.tensor.transpose(
                                    pt[:, bidx * 128:(bidx + 1) * 128]
                                    .bitcast(F32R),
                                    b1[q][:, off:off + 128], W(21))
                        drain(b2[q][:, wh * 1024:(wh + 1) * 1024], pt[:])

                # ---- pass C: contract d0,d1,d2 with c' (partitions (c', v012))
                # out partitions (c, e0e1e2); f already e3..e9 natural order.
                for q in range(8):
                    for uh in range(2):
                        pt = ps.tile([128, 1024], F32, tag="ps")
                        for ul in range(2):
                            u = 2 * uh + ul
                            nc.tensor.matmul(pt[:, ul * 512:(ul + 1) * 512],
                                             W(20),
                                             b2[q][:, u * 512:(u + 1) * 512],
                                             start=True, stop=True)
                        drain(b1[q][:, uh * 1024:(uh + 1) * 1024], pt[:])
                    eng = nc.sync if q % 2 == 0 else nc.scalar
                    eng.dma_start(xout_d[:, 2048 * q:2048 * (q + 1)],
                                  b1[q][:].bitcast(F32))


    nc.compile()
    return nc


def _get_nc():
    if "nc" not in _NC_CACHE:
        _NC_CACHE["nc"] = build_nc()
    return _NC_CACHE["nc"]


def pack_state(x_real, x_imag, b):
    """[DIM] re/im planes of batch item b -> [128, 16384] load layout L_A."""
    arr = np.stack([np.asarray(x_real[b], np.float32),
                    np.asarray(x_imag[b], np.float32)])        # [2, DIM]
    # i = v012*16384 + v345*256 + d6*64 + r789
    arr = arr.reshape(2, 64, 64, 4, 64).transpose(0, 2, 1, 3, 4)
    return np.ascontiguousarray(arr.reshape(128, 16384))


def unpack_state(xout):
    """[128, 16384] final layout -> ([DIM] re, [DIM] im)."""
    return xout[0:64].reshape(-1), xout[64:128].reshape(-1)


def make_runner(nc, n_cores=N_CORES):
    """Persistent sharded-jit callable for the compiled module (one jit trace,
    reused across kernel() calls)."""
    import jax
    from jax.sharding import Mesh, PartitionSpec
    from jax.experimental.shard_map import shard_map
    import concourse.mybir as mybir_
    from concourse.bass2jax import (_bass_exec_p, install_neuronx_cc_hook,
                                    partition_id_tensor)

    install_neuronx_cc_hook()
    part_name = (nc.partition_id_tensor.name
                 if nc.partition_id_tensor is not None else None)
    in_names, out_names, out_avals, zero_outs = [], [], [], []
    for alloc in nc.m.functions[0].allocations:
        if not isinstance(alloc, mybir_.MemoryLocationSet):
            continue
        name = alloc.memorylocations[0].name
        if alloc.kind == "ExternalInput":
            if name != part_name:
                in_names.append(name)
        elif alloc.kind == "ExternalOutput":
            shape = tuple(alloc.tensor_shape)
            dtype = mybir_.dt.np(alloc.dtype)
            out_names.append(name)
            out_avals.append(jax.core.ShapedArray(shape, dtype))
            zero_outs.append(np.zeros(shape, dtype))
    n_params = len(in_names)
    all_names = in_names + out_names
    if part_name is not None:
        all_names = all_names + [part_name]

    def _body(*args):
        operands = list(args)
        if part_name is not None:
            operands.append(partition_id_tensor())
        outs = _bass_exec_p.bind(
            *operands,
            out_avals=tuple(out_avals),
            in_names=tuple(all_names),
            out_names=tuple(out_names),
            lowering_input_output_aliases=(),
            sim_require_finite=True,
            sim_require_nnan=True,
            nc=nc,
        )
        return tuple(outs)

    devices = jax.devices()[:n_cores]
    mesh = Mesh(np.asarray(devices), ("core",))
    specs = (PartitionSpec("core"),) * (n_params + len(out_names))
    out_specs = (PartitionSpec("core"),) * len(out_names)
    fn = jax.jit(shard_map(_body, mesh=mesh, in_specs=specs,
                           out_specs=out_specs, check_rep=False),
                 keep_unused=True)

    def run(in_maps=None, concat_args=None):
        args = []
        if concat_args is not None:
            for name in in_names:
                args.append(np.asarray(concat_args[name]))
        else:
            for name in in_names:
                args.append(np.concatenate(
                    [np.asarray(m[name]) for m in in_maps], axis=0))
        for z in zero_outs:
            args.append(np.zeros((n_cores * z.shape[0], *z.shape[1:]),
                                 z.dtype))
        outs = fn(*args)
        return {name: np.asarray(outs[i]) for i, name in enumerate(out_names)}

    return run


def _get_runner():
    if "run" not in _NC_CACHE:
        _NC_CACHE["run"] = make_runner(_get_nc())
    return _NC_CACHE["run"]


def kernel(x_real, x_imag, u1_re, u1_im, u2_re, u2_im):
    run = _get_runner()
    wts = build_weights(u1_re, u1_im, u2_re, u2_im)

    # pack all 8 cores in one vectorized op:
    # core b partition (c*64 + v345), f = v012*256 + d6*64 + r789
    A = np.stack([np.asarray(x_real, np.float32),
                  np.asarray(x_imag, np.float32)], axis=1)     # [8, 2, DIM]
    A = A.reshape(BATCH, 2, 64, 64, 4, 64).transpose(0, 1, 3, 2, 4, 5)
    xin_all = np.ascontiguousarray(A.reshape(BATCH * 128, 16384))
    wts_all = np.tile(wts, (BATCH, 1))                         # [8*128, 2816]

    results = run(concat_args={"xin": xin_all, "wts": wts_all})
    xo = results["xout"].reshape(BATCH, 2, 64, 16384)          # [b, c, w012, f]
    out = xo.transpose(1, 0, 2, 3).reshape(2, BATCH, DIM)
    return np.ascontiguousarray(out)



# revision 3
# speedup vs baseline: 1.1090x; 1.1090x over previous
"""Trainium2 Bass kernel for nn_Circuit (batch quantum circuit simulation).

Math: the circuit is u1 on every qubit, u2 on disjoint adjacent pairs, u1 on
every qubit again.  Since all gates factor over the 10 disjoint qubit pairs,
the whole circuit equals M^{tensor 10} with M = (u1 x u1) @ u2 @ (u1 x u1),
a single 4x4 complex matrix applied to every base-4 digit of the state index.

Strategy (data-parallel, one batch item per core):
  State as real fp32 [128, 16384]; partition bits = (c | 3 digits) where c is
  the re/im component bit.  Three matmul passes with realified 128x128
  stationaries contract digits 3..6 (super-pass with d6 blocked in the free
  dim), 7..9, and 0..2; two full 128x128 PE-transpose exchanges move digit
  groups between partitions and free dim; c alternates partition/free and the
  middle pass mixes re/im via PSUM accumulation over component slices.
  All matmuls/transposes in fp32r (TF32); PSUM accumulation fp32.
"""
import numpy as np
import ml_dtypes

import concourse.bacc as bacc
import concourse.tile as tile
import concourse.mybir as mybir

F32 = mybir.dt.float32
F32R = mybir.dt.float32r
BF16 = mybir.dt.bfloat16

NQ = 20
BATCH = 8
DIM = 2 ** NQ
N_CORES = 8

_NC_CACHE = {}


def _realify(W):
    """lhsT [128,128] for out[(c',w)] = sum_{c,v} L[(c,v),(c',w)] x[(c,v)],
    complex W acting as out[w] = sum_v W[w,v] x[v]."""
    Wr, Wi = W.real, W.imag
    L = np.zeros((128, 128), np.float64)
    L[0:64, 0:64] = Wr.T
    L[64:128, 0:64] = -Wi.T
    L[0:64, 64:128] = Wi.T
    L[64:128, 64:128] = Wr.T
    return L


def build_weights(u1_re, u1_im, u2_re, u2_im):
    """Host-side stationaries, [128, 18*128] bf16."""
    u1 = np.asarray(u1_re, np.float64) + 1j * np.asarray(u1_im, np.float64)
    u2 = np.asarray(u2_re, np.float64) + 1j * np.asarray(u2_im, np.float64)
    A = np.kron(u1, u1)
    M = A @ u2 @ A                      # 4x4 complex
    W3 = np.kron(M, np.kron(M, M))      # 64x64 complex, digit-major

    mats = []
    # 0: realified W3 for pass A' (c,d345) and pass C (c',d012)
    mats.append(_realify(W3))
    # 1..16: pass B' stationaries kron(A2.T, C_ccp.T); A2 = M6 subblock
    for c in range(2):
        for x in range(2):          # d6h
            for cp in range(2):
                for eh in range(2):  # e6h
                    A2 = M[2 * eh:2 * eh + 2, 2 * x:2 * x + 2]
                    Kc = np.kron(A2.T, W3.T)
                    mats.append({(0, 0): Kc.real, (0, 1): Kc.imag,
                                 (1, 0): -Kc.imag, (1, 1): Kc.real}[(c, cp)])
    # 17: identity for PE transposes
    mats.append(np.eye(128))
    wts = np.stack(mats)                                  # [22,128,128]
    packed = wts.transpose(1, 0, 2).reshape(128, 18 * 128)
    return np.ascontiguousarray(packed).astype(ml_dtypes.bfloat16)


def build_nc(repeat=1):
    nc = bacc.Bacc("TRN2", target_bir_lowering=False, debug=False,
                   num_devices=N_CORES)
    xin_d = nc.dram_tensor("xin", [128, 16384], BF16, kind="ExternalInput").ap()
    wts_d = nc.dram_tensor("wts", [128, 18 * 128], BF16,
                           kind="ExternalInput").ap()
    xout_d = nc.dram_tensor("xout", [128, 16384], BF16,
                            kind="ExternalOutput").ap()

    dcnt = [0]

    with tile.TileContext(nc) as tc:
        with tc.tile_pool(name="sb", bufs=1) as sb, \
             tc.tile_pool(name="ps", bufs=4, space="PSUM") as ps:

            wt = sb.tile([128, 18 * 128], BF16, tag="wt")
            nc.gpsimd.dma_start(wt[:], wts_d)

            def W(m):
                return wt[:, m * 128:(m + 1) * 128]

            b1 = [sb.tile([128, 2048], BF16, tag=f"b1_{k}", name=f"b1_{k}")
                  for k in range(8)]
            b2 = [sb.tile([128, 2048], BF16, tag=f"b2_{k}", name=f"b2_{k}")
                  for k in range(8)]
            bC = sb.tile([128, 16384], BF16, tag="bC")

            for k in range(8):
                eng = nc.sync if k % 2 == 0 else nc.scalar
                eng.dma_start(b1[k][:], xin_d[:, 2048 * k:2048 * (k + 1)])

            def drain(o, i):
                if dcnt[0] % 2 == 0:
                    nc.scalar.copy(o, i)
                else:
                    nc.vector.tensor_copy(o, i)
                dcnt[0] += 1

            for _rep in range(repeat):
                # ---- pass A': contract (c, d3,d4,d5) only; layout unchanged
                for ck in range(8):
                    for ih in range(2):
                        pt = ps.tile([128, 1024], F32, tag="ps")
                        for il in range(2):
                            u = 2 * ih + il
                            nc.tensor.matmul(pt[:, il * 512:(il + 1) * 512],
                                             W(0),
                                             b1[ck][:, u * 512:(u + 1) * 512],
                                             start=True, stop=True)
                        drain(b2[ck][:, ih * 1024:(ih + 1) * 1024], pt[:])

                # ---- E1: full transposes; in-cols = (e6l, r789) contiguous 128
                # out -> bC layout L_C: f = w345*256 + e6h*128 + c*64 + v012
                bCo = bC[:].rearrange("p (w ec v) -> p v ec w",
                                      w=64, ec=4, v=64)
                for g in range(8):
                    for vh in range(2):
                        pt = ps.tile([128, 512], F32, tag="ps")
                        ptb = pt[:].bitcast(BF16)
                        for v4 in range(4):
                            v8 = 4 * vh + v4
                            for e6h in range(2):
                                bidx = v4 * 2 + e6h
                                off = v8 * 256 + e6h * 128
                                nc.tensor.transpose(
                                    ptb[:, bidx * 128:(bidx + 1) * 128],
                                    b2[g][:, off:off + 128], W(17))
                        inv = ptb[:].rearrange("p (v ec w) -> p v ec w",
                                               v=4, ec=4, w=64)
                        base = 8 * g + 4 * vh
                        drain(bCo[:, base:base + 4], inv)

                # ---- pass B': contract d789 gates + d6 (d6l on par,
                # d6h via slices) + complex (c via slices): 4-slice accum.
                # bC layout: f = w345*256 + d6h*128 + c*64 + v012
                bCv = bC[:].rearrange("p (t w x c v) -> p x c t w v",
                                      t=8, w=8, x=2, c=2, v=64)
                for t8 in range(8):
                    outq = b1[t8][:].rearrange("p (w x c v) -> p c x w v",
                                               w=8, x=2, c=2, v=64)
                    for cpp in range(2):
                        pt = ps.tile([128, 1024], F32, tag="ps")
                        for eh in range(2):
                            si = 0
                            for c in range(2):
                                for x in range(2):
                                    m = 1 + ((c * 2 + x) * 2 + cpp) * 2 + eh
                                    nc.tensor.matmul(
                                        pt[:, eh * 512:(eh + 1) * 512],
                                        W(m), bCv[:, x, c, t8],
                                        start=(si == 0), stop=(si == 3))
                                    si += 1
                        inv = pt[:].rearrange("p (x w v) -> p x w v",
                                              x=2, w=8, v=64)
                        drain(outq[:, cpp], inv)

                # ---- E2: full transposes; in-cols = (c', v012) contiguous 128
                # out -> b2 layout L_E: f = w345*256 + e6*64 + w789 (plain copy)
                for q in range(8):
                    for wh in range(2):
                        pt = ps.tile([128, 512], F32, tag="ps")
                        ptb = pt[:].bitcast(BF16)
                        for w4 in range(4):
                            for e6h in range(2):
                                bidx = w4 * 2 + e6h
                                off = (4 * wh + w4) * 256 + e6h * 128
                                nc.tensor.transpose(
                                    ptb[:, bidx * 128:(bidx + 1) * 128],
                                    b1[q][:, off:off + 128], W(17))
                        drain(b2[q][:, wh * 1024:(wh + 1) * 1024], ptb[:])

                # ---- pass C: contract d0,d1,d2 with c' (partitions (c', v012))
                # out partitions (c, e0e1e2); f already e3..e9 natural order.
                for q in range(8):
                    for uh in range(2):
                        pt = ps.tile([128, 1024], F32, tag="ps")
                        for ul in range(2):
                            u = 2 * uh + ul
                            nc.tensor.matmul(pt[:, ul * 512:(ul + 1) * 512],
                                             W(0),
                                             b2[q][:, u * 512:(u + 1) * 512],
                                             start=True, stop=True)
                        drain(b1[q][:, uh * 1024:(uh + 1) * 1024], pt[:])
                    eng = nc.sync if q % 2 == 0 else nc.scalar
                    eng.dma_start(xout_d[:, 2048 * q:2048 * (q + 1)],
                                  b1[q][:])


    nc.compile()
    return nc


def _get_nc():
    if "nc" not in _NC_CACHE:
        _NC_CACHE["nc"] = build_nc()
    return _NC_CACHE["nc"]


def pack_state(x_real, x_imag, b):
    """[DIM] re/im planes of batch item b -> [128, 16384] load layout L_A."""
    arr = np.stack([np.asarray(x_real[b], np.float32),
                    np.asarray(x_imag[b], np.float32)])        # [2, DIM]
    # i = v012*16384 + v345*256 + d6*64 + r789
    arr = arr.reshape(2, 64, 64, 4, 64).transpose(0, 2, 1, 3, 4)
    return np.ascontiguousarray(arr.reshape(128, 16384)).astype(ml_dtypes.bfloat16)


def unpack_state(xout):
    """[128, 16384] final layout -> ([DIM] re, [DIM] im)."""
    return xout[0:64].reshape(-1), xout[64:128].reshape(-1)


def make_runner(nc, n_cores=N_CORES):
    """Persistent sharded-jit callable for the compiled module (one jit trace,
    reused across kernel() calls)."""
    import jax
    from jax.sharding import Mesh, PartitionSpec
    from jax.experimental.shard_map import shard_map
    import concourse.mybir as mybir_
    from concourse.bass2jax import (_bass_exec_p, install_neuronx_cc_hook,
                                    partition_id_tensor)

    install_neuronx_cc_hook()
    part_name = (nc.partition_id_tensor.name
                 if nc.partition_id_tensor is not None else None)
    in_names, out_names, out_avals, zero_outs = [], [], [], []
    for alloc in nc.m.functions[0].allocations:
        if not isinstance(alloc, mybir_.MemoryLocationSet):
            continue
        name = alloc.memorylocations[0].name
        if alloc.kind == "ExternalInput":
            if name != part_name:
                in_names.append(name)
        elif alloc.kind == "ExternalOutput":
            shape = tuple(alloc.tensor_shape)
            dtype = mybir_.dt.np(alloc.dtype)
            out_names.append(name)
            out_avals.append(jax.core.ShapedArray(shape, dtype))
            zero_outs.append(np.zeros(shape, dtype))
    n_params = len(in_names)
    all_names = in_names + out_names
    if part_name is not None:
        all_names = all_names + [part_name]

    def _body(*args):
        operands = list(args)
        if part_name is not None:
            operands.append(partition_id_tensor())
        outs = _bass_exec_p.bind(
            *operands,
            out_avals=tuple(out_avals),
            in_names=tuple(all_names),
            out_names=tuple(out_names),
            lowering_input_output_aliases=(),
            sim_require_finite=True,
            sim_require_nnan=True,
            nc=nc,
        )
        return tuple(outs)

    devices = jax.devices()[:n_cores]
    mesh = Mesh(np.asarray(devices), ("core",))
    specs = (PartitionSpec("core"),) * (n_params + len(out_names))
    out_specs = (PartitionSpec("core"),) * len(out_names)
    fn = jax.jit(shard_map(_body, mesh=mesh, in_specs=specs,
                           out_specs=out_specs, check_rep=False),
                 keep_unused=True)

    def run(in_maps=None, concat_args=None):
        args = []
        if concat_args is not None:
            for name in in_names:
                args.append(np.asarray(concat_args[name]))
        else:
            for name in in_names:
                args.append(np.concatenate(
                    [np.asarray(m[name]) for m in in_maps], axis=0))
        for z in zero_outs:
            args.append(np.zeros((n_cores * z.shape[0], *z.shape[1:]),
                                 z.dtype))
        outs = fn(*args)
        return {name: np.asarray(outs[i]) for i, name in enumerate(out_names)}

    return run


def _get_runner():
    if "run" not in _NC_CACHE:
        _NC_CACHE["run"] = make_runner(_get_nc())
    return _NC_CACHE["run"]


def kernel(x_real, x_imag, u1_re, u1_im, u2_re, u2_im):
    run = _get_runner()
    wts = build_weights(u1_re, u1_im, u2_re, u2_im)

    # pack all 8 cores in one vectorized op:
    # core b partition (c*64 + v345), f = v012*256 + d6*64 + r789
    A = np.stack([np.asarray(x_real, np.float32),
                  np.asarray(x_imag, np.float32)], axis=1)     # [8, 2, DIM]
    A = A.reshape(BATCH, 2, 64, 64, 4, 64).transpose(0, 1, 3, 2, 4, 5)
    xin_all = np.ascontiguousarray(
        A.reshape(BATCH * 128, 16384)).astype(ml_dtypes.bfloat16)
    wts_all = np.tile(wts, (BATCH, 1))                         # [8*128, 2816]

    results = run(concat_args={"xin": xin_all, "wts": wts_all})
    xo = np.asarray(results["xout"], np.float32).reshape(BATCH, 2, 64, 16384)          # [b, c, w012, f]
    out = xo.transpose(1, 0, 2, 3).reshape(2, BATCH, DIM)
    return np.ascontiguousarray(out)

